# revision 6
# baseline (speedup 1.0000x reference)
"""Trainium2 Bass kernel for a 3-layer edge-weighted GCN (graph message
passing), distributed over 8 NeuronCores.

Strategy (graph/data parallel, per the sharding hint):
  - Nodes are partitioned into 8 contiguous ranges of 6250; core c owns the
    edges whose dst lands in its range (so each core produces the final rows
    for its own node range with no cross-core reduction).
  - Algebraic reorder: reference computes segsum(w_e * (x@W)[src]); we
    compute segsum(w_e * x[src]) @ W  (exact, biases are zero per spec),
    so the dense matmul runs on the dst-sharded aggregate and only raw
    node features ever cross cores.
  - Aggregation on device: edges sorted by dst window (128 nodes); for each
    128-edge chunk, gather x[src] rows with dma_gather (SWDGE), build a
    w-scaled one-hot [edge, dst-offset] matrix with one dual-op DVE
    instruction (is_equal then mult), and accumulate
    psum[f, n] += m[e, f]^T @ onehot[e, n] on the TensorEngine. The psum
    window is the transposed aggregate, which feeds the layer matmul
    directly as the stationary operand.
  - Node features are stored/exchanged in bf16 (fp32 accumulation in PSUM);
    layer boundaries replicate the new features with an AllGather.
  - dma_gather indices are int16, so the 50176-row node table is split into
    two 25088-row halves (= ranks 0-3 / 4-7); every (window, half) edge
    group is padded to whole 128-edge chunks with w=0 edges, with a chunk
    count shared by all 8 cores so one SPMD program fits every core.
"""
import numpy as np
import ml_dtypes

import concourse.bass as bass
import concourse.bacc as bacc
import concourse.mybir as mybir
import concourse.tile as tile
from concourse.bass_utils import run_bass_kernel_spmd

# problem shape (hardcoded per spec nn_GCNModel_18073222381931)
N_NODES = 50000
N_EDGES = 500000
F = 128          # in feats
HID = 128        # hidden
OUT = 64         # classes
NCORES = 8

P = 128
NPC = N_NODES // NCORES            # 6250 nodes per core
NWIN = (NPC + P - 1) // P          # 49 windows of 128 dst nodes
NPAD = NWIN * P                    # 6272 padded nodes per core
NTOT = NCORES * NPAD               # 50176 padded node table rows
HALF = (NCORES // 2) * NPAD        # 25088 (int16-indexable halves)

GW = 8                             # windows per dma_gather batch

bf16 = mybir.dt.bfloat16
f32 = mybir.dt.float32
bfnp = ml_dtypes.bfloat16


def _wrap_idx(idx_flat):
    """dma_gather index layout: edge i -> [i%16, i//16], replicated across
    the 8 Q7 partition groups."""
    n = len(idx_flat)
    assert n % 128 == 0
    w = idx_flat.reshape(n // 16, 16).T.astype(np.int16)   # [16, n//16]
    return np.ascontiguousarray(np.tile(w, (8, 1)))        # [128, n//16]


def prep(x, src, dst, w1, w2, w3):
    """Host-side sharding/index prep. Returns (structure, in_maps)."""
    src = np.asarray(src).astype(np.int64)
    dst = np.asarray(dst).astype(np.int64)
    ws = [np.asarray(w, np.float32) for w in (w1, w2, w3)]

    src_pid = (src // NPC) * NPAD + (src % NPC)   # padded node id
    core = dst // NPC
    loc = dst % NPC
    win = loc // P
    doff = (loc % P).astype(np.float32)
    half = (src_pid >= HALF).astype(np.int64)

    # chunk counts per (window, half), shared across cores (SPMD)
    cnt = np.zeros((NCORES, NWIN, 2), np.int64)
    np.add.at(cnt, (core, win, half), 1)
    nch = -(-cnt.max(axis=0) // P)                # [NWIN, 2] ceil
    for w in range(NWIN):
        if nch[w].sum() == 0:
            nch[w, 0] = 1
    ncha = int(nch[:, 0].sum())
    nchb = int(nch[:, 1].sum())
    ncht = ncha + nchb

    # global chunk index layout: per window, A chunks then B chunks
    chunk_base = np.zeros((NWIN, 2), np.int64)
    run = 0
    for w in range(NWIN):
        chunk_base[w, 0] = run
        run += nch[w, 0]
        chunk_base[w, 1] = run
        run += nch[w, 1]
    assert run == ncht
    epad = ncht * P

    # chunk -> half flag, and A/B-local chunk numbering
    chunk_half = np.zeros(ncht, np.int64)
    for w in range(NWIN):
        chunk_half[chunk_base[w, 1]:chunk_base[w, 1] + nch[w, 1]] = 1
    a_cols = np.nonzero(chunk_half == 0)[0]       # global chunk -> A list pos
    b_cols = np.nonzero(chunk_half == 1)[0]

    # per-core padded edge arrays in global chunk order
    gsrc = np.zeros((NCORES, epad), np.int64)     # padded node id (0 = pad)
    dofa = np.zeros((NCORES, epad), np.float32)
    wfa = np.zeros((3, NCORES, epad), np.float32)
    # pad entries in B chunks must index the B table: point at row HALF
    for w in range(NWIN):
        s = chunk_base[w, 1] * P
        e = s + nch[w, 1] * P
        gsrc[:, s:e] = HALF

    order = np.lexsort((half, win, core))
    so_core = core[order]
    so_win = win[order]
    so_half = half[order]
    so_src = src_pid[order]
    so_doff = doff[order]
    so_w = [w[order] for w in ws]
    # position within each (core, win, half) group
    keys = (so_core * NWIN * 2 + so_win * 2 + so_half)
    startmask = np.ones(len(keys), bool)
    startmask[1:] = keys[1:] != keys[:-1]
    gstart = np.nonzero(startmask)[0]
    within = np.arange(len(keys)) - np.repeat(
        gstart, np.diff(np.append(gstart, len(keys))))
    pos = chunk_base[so_win, so_half] * P + within
    gsrc[so_core, pos] = so_src
    dofa[so_core, pos] = so_doff
    for i in range(3):
        wfa[i, so_core, pos] = so_w[i]

    # transposed [P, ncht] layouts (edge k*128+p -> [p, k])
    def tr(a):
        return np.ascontiguousarray(a.reshape(ncht, P).T)

    xp = np.zeros((NTOT, F), bfnp)
    xf = np.asarray(x, np.float32)
    for c in range(NCORES):
        xp[c * NPAD:c * NPAD + NPC] = xf[c * NPC:(c + 1) * NPC].astype(bfnp)

    iota = np.broadcast_to(np.arange(P, dtype=np.float32), (P, P))

    in_maps = []
    for c in range(NCORES):
        idx_a = gsrc[c].reshape(ncht, P)[chunk_half == 0].ravel()
        idx_b = gsrc[c].reshape(ncht, P)[chunk_half == 1].ravel() - HALF
        if len(idx_b) == 0:
            idx_b = np.zeros(P, np.int64)
        in_maps.append({
            "xpA": np.ascontiguousarray(xp[:HALF]),
            "xpB": np.ascontiguousarray(xp[HALF:]),
            "idxA": _wrap_idx(idx_a),
            "idxB": _wrap_idx(idx_b),
            "doff": tr(dofa[c]),
            "wv1": tr(wfa[0, c]),
            "wv2": tr(wfa[1, c]),
            "wv3": tr(wfa[2, c]),
            "iota": np.ascontiguousarray(iota.astype(bfnp)),
        })

    struct = {
        "nch": nch, "chunk_base": chunk_base, "ncha": ncha, "nchb": nchb,
        "ncht": ncht, "a_cols": a_cols, "b_cols": b_cols,
    }
    return struct, in_maps


def build(struct, repeat=1):
    """Build the SPMD Bass program (weights arrive as ExternalInputs)."""
    nch = struct["nch"]
    ncha, nchb, ncht = struct["ncha"], struct["nchb"], struct["ncht"]
    a_of = {}
    b_of = {}
    # map global chunk -> position in A/B gather streams
    a_list = list(struct["a_cols"])
    b_list = list(struct["b_cols"])
    for i, g in enumerate(a_list):
        a_of[g] = i
    for i, g in enumerate(b_list):
        b_of[g] = i

    nc = bacc.Bacc("TRN2", target_bir_lowering=False, debug=False,
                   num_devices=NCORES)
    xpA = nc.dram_tensor("xpA", [HALF, F], bf16, kind="ExternalInput")
    xpB = nc.dram_tensor("xpB", [HALF, F], bf16, kind="ExternalInput")
    idxA = nc.dram_tensor("idxA", [P, ncha * 8], mybir.dt.int16,
                          kind="ExternalInput")
    idxB = nc.dram_tensor("idxB", [P, max(nchb, 1) * 8], mybir.dt.int16,
                          kind="ExternalInput")
    doff = nc.dram_tensor("doff", [P, ncht], f32, kind="ExternalInput")
    wvs = [nc.dram_tensor(f"wv{i+1}", [P, ncht], f32, kind="ExternalInput")
           for i in range(3)]
    iota = nc.dram_tensor("iota", [P, P], bf16, kind="ExternalInput")
    Wd = [nc.dram_tensor("W1", [F, HID], bf16, kind="ExternalInput"),
          nc.dram_tensor("W2", [HID, HID], bf16, kind="ExternalInput"),
          nc.dram_tensor("W3", [HID, OUT], bf16, kind="ExternalInput")]
    out = nc.dram_tensor("out", [NPAD, OUT], f32, kind="ExternalOutput")
    hpart = [nc.dram_tensor(f"hpart{l}", [NPAD, HID], bf16) for l in range(2)]
    hfull = [nc.dram_tensor(f"hfull{l}", [NTOT, HID], bf16,
                            addr_space="Shared") for l in range(2)]
    hfullB = [nc.dram_tensor(f"hfullB{l}", [HALF, HID], bf16)
              for l in range(2)]

    with tile.TileContext(nc) as tc:
        with (
            tc.tile_pool(name="const", bufs=1) as cst,
            tc.tile_pool(name="ma", bufs=3) as map_,
            tc.tile_pool(name="mb", bufs=3) as mbp,
            tc.tile_pool(name="oh", bufs=8) as ohp,
            tc.tile_pool(name="agg", bufs=4) as aggp,
            tc.tile_pool(name="ho", bufs=4) as hop,
            tc.tile_pool(name="psa", bufs=4, space="PSUM") as psa,
            tc.tile_pool(name="pso", bufs=2, space="PSUM") as pso,
        ):
            idxA_sb = cst.tile([P, ncha * 8], mybir.dt.int16, tag="idxA")
            idxB_sb = cst.tile([P, max(nchb, 1) * 8], mybir.dt.int16,
                               tag="idxB")
            doff_sb = cst.tile([P, ncht], f32, tag="doff")
            wv_sb = [cst.tile([P, ncht], f32, tag=f"wv{i}",
                              name=f"wv{i}_sb") for i in range(3)]
            iota_sb = cst.tile([P, P], bf16, tag="iota")
            W_sb = [cst.tile([F, HID], bf16, tag="W1", name="W1_sb"),
                    cst.tile([HID, HID], bf16, tag="W2", name="W2_sb"),
                    cst.tile([HID, OUT], bf16, tag="W3", name="W3_sb")]
            nc.sync.dma_start(out=idxA_sb[:], in_=idxA[:, :])
            nc.sync.dma_start(out=idxB_sb[:], in_=idxB[:, :])
            nc.sync.dma_start(out=doff_sb[:], in_=doff[:, :])
            for i in range(3):
                nc.sync.dma_start(out=wv_sb[i][:], in_=wvs[i][:, :])
                nc.sync.dma_start(out=W_sb[i][:], in_=Wd[i][:, :])
            nc.sync.dma_start(out=iota_sb[:], in_=iota[:, :])

            wgroups = [list(range(g, min(g + GW, NWIN)))
                       for g in range(0, NWIN, GW)]

            for _ in range(repeat):
                for l in range(3):
                    tabA = (xpA[:, :] if l == 0
                            else hfull[l - 1][0:HALF, :])
                    tabB = (xpB[:, :] if l == 0
                            else hfullB[l - 1][:, :])
                    outf = HID if l < 2 else OUT
                    for grp in wgroups:
                        ga = [struct["chunk_base"][w, 0] + i
                              for w in grp for i in range(nch[w, 0])]
                        gb = [struct["chunk_base"][w, 1] + i
                              for w in grp for i in range(nch[w, 1])]
                        kA, kB = len(ga), len(gb)
                        a0 = a_of[ga[0]] if kA else 0
                        b0 = b_of[gb[0]] if kB else 0
                        mAt = mBt = None
                        if kA:
                            mAt = map_.tile([P, kA, F], bf16, tag="mA")
                            nc.gpsimd.dma_gather(
                                out_ap=mAt[:],
                                in_ap=tabA,
                                idxs_ap=idxA_sb[:, a0 * 8:(a0 + kA) * 8],
                                num_idxs=kA * P,
                                num_idxs_reg=kA * P,
                                elem_size=F,
                                single_packet=False,
                            )
                        if kB:
                            mBt = mbp.tile([P, kB, F], bf16, tag="mB")
                            nc.gpsimd.dma_gather(
                                out_ap=mBt[:],
                                in_ap=tabB,
                                idxs_ap=idxB_sb[:, b0 * 8:(b0 + kB) * 8],
                                num_idxs=kB * P,
                                num_idxs_reg=kB * P,
                                elem_size=F,
                                single_packet=False,
                            )
                        for w in grp:
                            chunks = []
                            for i in range(nch[w, 0]):
                                g = struct["chunk_base"][w, 0] + i
                                chunks.append((mAt, a_of[g] - a0, g))
                            for i in range(nch[w, 1]):
                                g = struct["chunk_base"][w, 1] + i
                                chunks.append((mBt, b_of[g] - b0, g))
                            pa = psa.tile([P, P], f32, tag="pa")
                            for j, (mt, lc, g) in enumerate(chunks):
                                oh = ohp.tile([P, P], bf16, tag="oh")
                                nc.vector.tensor_scalar(
                                    out=oh[:],
                                    in0=iota_sb[:],
                                    scalar1=doff_sb[:, g:g + 1],
                                    scalar2=wv_sb[l][:, g:g + 1],
                                    op0=mybir.AluOpType.is_equal,
                                    op1=mybir.AluOpType.mult,
                                )
                                nc.tensor.matmul(
                                    pa[:], lhsT=mt[:, lc, :], rhs=oh[:],
                                    start=(j == 0),
                                    stop=(j == len(chunks) - 1))
                            aggT = aggp.tile([P, P], bf16, tag="aggT")
                            nc.vector.tensor_copy(out=aggT[:], in_=pa[:])
                            po = pso.tile([P, outf], f32, tag="po")
                            nc.tensor.matmul(po[:], lhsT=aggT[:],
                                             rhs=W_sb[l][:, :],
                                             start=True, stop=True)
                            if l < 2:
                                ht = hop.tile([P, HID], bf16, tag="ht")
                                nc.scalar.activation(
                                    ht[:], po[:],
                                    mybir.ActivationFunctionType.Relu)
                                nc.sync.dma_start(
                                    out=hpart[l][w * P:(w + 1) * P, :],
                                    in_=ht[:])
                            else:
                                ot = hop.tile([P, OUT], f32, tag="ot")
                                nc.vector.tensor_copy(out=ot[:], in_=po[:])
                                nc.sync.dma_start(
                                    out=out[w * P:(w + 1) * P, :],
                                    in_=ot[:])
                    if l < 2:
                        nc.gpsimd.collective_compute(
                            "AllGather",
                            mybir.AluOpType.bypass,
                            replica_groups=[list(range(NCORES))],
                            ins=[hpart[l][:, :]],
                            outs=[hfull[l][:, :]],
                        )
                        nc.sync.dma_start(out=hfullB[l][:, :],
                                          in_=hfull[l][HALF:NTOT, :])
    nc.compile()
    return nc


def _to_in_maps(in_maps, Wmats):
    W1, W2, W3 = Wmats
    for m in in_maps:
        m["W1"] = np.asarray(W1, np.float32).astype(bfnp)
        m["W2"] = np.asarray(W2, np.float32).astype(bfnp)
        m["W3"] = np.asarray(W3, np.float32).astype(bfnp)
    return in_maps


def kernel(x, src, dst, w1, w2, w3, W1, b1, W2, b2, W3, b3, _repeat=1,
           _prebuilt=None):
    # biases are zero by construction (spec fill=zeros)
    if np.any(b1) or np.any(b2) or np.any(b3):
        raise NotImplementedError("nonzero biases not supported")
    struct, in_maps = prep(x, src, dst, w1, w2, w3)
    in_maps = _to_in_maps(in_maps, (W1, W2, W3))
    nc = _prebuilt or build(struct, repeat=_repeat)
    res = run_bass_kernel_spmd(nc, in_maps, list(range(NCORES)))
    outs = [res.results[c]["out"][:NPC] for c in range(NCORES)]
    return np.concatenate(outs, axis=0).astype(np.float32)


if __name__ == "__main__":
    pass
